# revision 36
# baseline (speedup 1.0000x reference)
"""Trainium2 Bass kernel: single-head attention with QKV+output projections.

Reference math (B=4, S=4096, D=64):
    Q = q@Wq.T+bq; K = k@Wk.T+bk; V = v@Wv.T+bv
    s = (Q @ K.T) / 8, masked -inf where i > j  (query i attends keys j >= i)
    out = softmax(s) @ V @ Wp.T + bp

Sharding (8 cores): core c -> batch b = c//2, parity h = c%2.
Each core handles 16 query tiles of 128 rows: global tiles t = 2s+h
(s = local slot 0..15). For key-block kb (128 keys), the valid local
query prefix is Nq(kb) = 128*min(kb//2+1, 16) columns (slot s valid iff
t <= kb). Scores are computed transposed (S^T: keys on partitions,
queries on free dim) so the PV matmul needs no transposes; the softmax
denominator l rides along as a ones-column appended to V''=V@Wp.T.
Masking of the one partially-valid/invalid last slot per key-block is
data-driven (per-core {0,1} bf16 mask tensors), so all 8 cores share a
single SPMD graph. No max-subtraction in softmax: |s/8| < ~3, exp is
safe in fp32.

All TensorEngine inputs are bf16 (fp32/f32r matmuls and PE transposes
interact badly on HW); PSUM accumulation stays fp32. V-hat natural
layout comes from DMA-transposes (XBAR), not PE transposes. The final
normalize runs in the O''^T layout: 1/l is computed compactly and
row-broadcast via a rank-1 bf16 matmul; the host transposes the
[64, 2048] result back (pure data movement).
"""

import numpy as np

import concourse.bass as bass
import concourse.mybir as mybir
from concourse.bass_utils import run_bass_kernel_spmd

B, S, D = 4, 4096, 64
NSLOT = 16  # query tiles (slots) per core
NKB = 32  # key blocks of 128
QL = NSLOT * 128  # local queries per core = 2048
VST = 80  # vhat block stride (65 used; multiple of 16 for XBAR dma-transpose)
NWARM = 48  # dummy warm-up matmuls to unthrottle the PE HAM clock gate

FP = mybir.dt.float32
BF = mybir.dt.bfloat16

# prep fills: [64,1024] PSUM projection fills; copies alternate ACT/DVE
FILLS = (
    [("q", 1024 * f) for f in range(2)]
    + [("k", 1024 * f) for f in range(4)]
    + [("v", 1024 * f) for f in range(4)]
)


def nq_of(kb: int) -> int:
    """Valid local-query prefix width (cols of S^T) at key-block kb."""
    if kb < 0:
        return 0
    return 128 * min(kb // 2 + 1, NSLOT)


def chunks(lo: int, hi: int, step: int):
    out = []
    c = lo
    while c < hi:
        out.append((c, min(c + step, hi)))
        c = out[-1][1]
    return out


def make_units():
    units = []
    for kb in range(NKB):
        w = nq_of(kb)
        if w <= 1024:
            units.append((kb, 0, w))
        else:
            units.append((kb, 0, 1024))
            units.append((kb, 1024, w))
    return units


def make_plan():
    """Precompute per-engine cumulative instruction counts at key events."""
    units = make_units()
    p = {
        "units": units,
        "pe_after_fill": {},
        "pe_after_sunit": {},
        "pe_after_pv": {},
        "act_after_fillcopy": {},
        "dve_after_fillcopy": {},
        "act_after_unit": {},
        "dve_after_mask": {},
    }

    # --- PE --- HAM warm-up burst, fills, then S/PV loop, bcast matmuls
    pe = NWARM
    for fi in range(len(FILLS)):
        pe += 2
        p["pe_after_fill"][fi] = pe
    for b in range(NKB):
        pe += 1
        p.setdefault("pe_after_vnat", {})[b] = pe
    for u, (kb, lo, hi) in enumerate(units):
        pe += len(chunks(lo, hi, 512))
        p["pe_after_sunit"][u] = pe
        if (u + 1 == len(units)) or units[u + 1][0] != kb:
            pv = kb - 1
            if pv >= 0:
                pe += len(chunks(0, nq_of(pv), 512))
                p["pe_after_pv"][pv] = pe
    pe += len(chunks(0, nq_of(NKB - 1), 512))
    p["pe_after_pv"][NKB - 1] = pe

    # --- ACT --- even fill copies, exps, l-row copy
    act = 0
    for fi in range(0, len(FILLS), 2):
        act += 1
        p["act_after_fillcopy"][fi] = act
    for u in range(len(units)):
        act += 1
        p["act_after_unit"][u] = act
    act += 1
    p["act_after_lrow"] = act
    act += 1  # ACT reciprocal of the broadcast l
    p["act_after_recip"] = act

    # --- DVE --- memsets, odd fill copies, vhat copies, p_acc memset, masks
    dve = 3  # warmup-scratch memset + vhat-ones memset + ones164 memset
    for fi in range(1, len(FILLS), 2):
        dve += 1
        p["dve_after_fillcopy"][fi] = dve
    for b in range(NKB):
        dve += 1
        p.setdefault("dve_after_vcopy", {})[b] = dve
    dve += 1  # p_acc zero-init
    p["dve_acc_memset"] = dve
    for kb in range(NKB):
        dve += 1
        p["dve_after_mask"][kb] = dve
    dve += 1  # l row f32 -> bf16 cast
    p["dve_after_lcast"] = dve
    dve += 2  # stt, bp add
    p["dve_after_y"] = dve

    return p


def fillcopy_wait(plan, fi):
    """(sem_name, threshold) completing the copy-out of fill fi."""
    if fi % 2 == 0:
        return ("act", plan["act_after_fillcopy"][fi])
    return ("dve", plan["dve_after_fillcopy"][fi])


def build():
    plan = make_plan()
    units = plan["units"]

    nc = bass.Bass()

    # ---- DRAM parameters (per-core shards, host-prepared, bf16) ----
    d_qta = nc.declare_dram_parameter("qta", [65, QL], BF, isOutput=False)
    d_kta = nc.declare_dram_parameter("kta", [65, S], BF, isOutput=False)
    d_vta = nc.declare_dram_parameter("vta", [65, S], BF, isOutput=False)
    d_wq = nc.declare_dram_parameter("wq", [65, 64], BF, isOutput=False)
    d_wk = nc.declare_dram_parameter("wk", [65, 64], BF, isOutput=False)
    d_wv = nc.declare_dram_parameter("wv", [65, 64], BF, isOutput=False)
    d_wp = nc.declare_dram_parameter("wp", [64, 64], BF, isOutput=False)
    d_bpc = nc.declare_dram_parameter("bpc", [64, 1], FP, isOutput=False)
    d_me = nc.declare_dram_parameter("me", [128, 128], BF, isOutput=False)
    d_mo = nc.declare_dram_parameter("mo", [128, 128], BF, isOutput=False)
    d_o = nc.declare_dram_parameter("o", [64, QL], FP, isOutput=True)


    # ---- persistent SBUF ----
    s_qta = nc.alloc_sbuf_tensor("s_qta", [65, QL], BF)
    s_kta = nc.alloc_sbuf_tensor("s_kta", [65, S], BF)
    s_vta = nc.alloc_sbuf_tensor("s_vta", [65, S], BF)
    s_wq = nc.alloc_sbuf_tensor("s_wq", [65, 64], BF)
    s_wk = nc.alloc_sbuf_tensor("s_wk", [65, 64], BF)
    s_wv = nc.alloc_sbuf_tensor("s_wv", [65, 64], BF)
    s_wp = nc.alloc_sbuf_tensor("s_wp", [64, 64], BF)
    s_bpc = nc.alloc_sbuf_tensor("s_bpc", [64, 1], FP)
    s_me = nc.alloc_sbuf_tensor("s_me", [128, 128], BF)
    s_mo = nc.alloc_sbuf_tensor("s_mo", [128, 128], BF)
    s_QT = nc.alloc_sbuf_tensor("s_QT", [64, QL], BF)
    s_KT = nc.alloc_sbuf_tensor("s_KT", [64, S], BF)
    s_VT = nc.alloc_sbuf_tensor("s_VT", [64, S], BF)
    s_vhat = nc.alloc_sbuf_tensor("s_vhat", [128, NKB * VST], BF)
    s_P = [
        nc.alloc_sbuf_tensor("s_P0", [128, QL], BF),
        nc.alloc_sbuf_tensor("s_P1", [128, QL], BF),
    ]
    s_lrow = nc.alloc_sbuf_tensor("s_lrow", [1, QL], FP)
    s_lrowb = nc.alloc_sbuf_tensor("s_lrowb", [1, QL], BF)
    s_scr = nc.alloc_sbuf_tensor("s_scr", [64, QL], FP)
    s_ones1 = nc.alloc_sbuf_tensor("s_ones1", [1, 64], BF)
    s_rb = nc.alloc_sbuf_tensor("s_rb", [64, QL], FP)
    s_Y = nc.alloc_sbuf_tensor("s_Y", [64, QL], FP)
    s_wu = nc.alloc_sbuf_tensor("s_wu", [64, 640], BF)

    proj_dst = {"q": s_QT, "k": s_KT, "v": s_VT}

    with (
        nc.semaphore("sem_dq") as sem_dq,
        nc.semaphore("sem_dk") as sem_dk,
        nc.semaphore("sem_dv") as sem_dv,
        nc.semaphore("sem_dw") as sem_dw,
        nc.semaphore("sem_dm") as sem_dm,
        nc.semaphore("sem_pe") as sem_pe,
        nc.semaphore("sem_act") as sem_act,
        nc.semaphore("sem_dve") as sem_dve,
        nc.semaphore("sem_out") as sem_out,
    ):
        sems = {"act": sem_act, "dve": sem_dve}

        # =============== input DMAs ===============
        with nc.Block() as blk:

            @blk.sync
            def _(sync):
                groups = [
                    (sem_dq, [(d_wq, s_wq), (d_qta, s_qta)]),
                    (sem_dk, [(d_wk, s_wk), (d_kta, s_kta)]),
                    (sem_dv, [(d_wv, s_wv), (d_vta, s_vta)]),
                    (sem_dw, [(d_wp, s_wp)]),
                    (sem_dm, [(d_bpc, s_bpc), (d_me, s_me), (d_mo, s_mo)]),
                ]
                for sem, pairs in groups:
                    for dsrc, sdst in pairs:
                        sync.dma_start(sdst[:, :], dsrc[:, :]).then_inc(sem, 16)

        # =============== prep stage ===============
        with (
            nc.psum_tensor("pA", [64, 1024], FP) as pA,
            nc.psum_tensor("pB", [64, 1024], FP) as pB,
            nc.psum_tensor("pV", [128, 1024], FP) as pV,
        ):
            pbuf = [pA, pB]

            with nc.Block() as blk:

                @blk.tensor
                def _(tensor):
                    # HAM warm-up: ~10us of back-to-back matmuls on scratch
                    # data while the input DMAs are in flight. Keeps the PE
                    # activity window busy so the clock gate opens (1.2 ->
                    # 2.4 GHz) before the real work arrives.
                    tensor.wait_ge(sem_dve, 1)
                    for _ in range(NWARM):
                        tensor.matmul(
                            pA[0:64, 0:512],
                            s_wu[0:64, 0:64],
                            s_wu[0:64, 128:640],
                            start=True,
                            stop=True,
                        ).then_inc(sem_pe, 1)
                    for fi, (kind, lo) in enumerate(FILLS):
                        buf = pbuf[fi % 2]
                        if fi >= 2:
                            sname, thr = fillcopy_wait(plan, fi - 2)
                            tensor.wait_ge(sems[sname], thr)
                        if kind == "q":
                            tensor.wait_ge(sem_dq, 32)
                            w_ap, src, cdim = s_wq, s_qta, 65
                        elif kind == "k":
                            tensor.wait_ge(sem_dk, 32)
                            w_ap, src, cdim = s_wk, s_kta, 65
                        else:
                            tensor.wait_ge(sem_dv, 32)
                            w_ap, src, cdim = s_wv, s_vta, 65
                        for c0, c1 in chunks(lo, lo + 1024, 512):
                            tensor.matmul(
                                buf[0:64, c0 - lo : c1 - lo],
                                w_ap[0:cdim, 0:64],
                                src[0:cdim, c0:c1],
                                start=True,
                                stop=True,
                            ).then_inc(sem_pe, 1)

                    # V''-natural tiles: lhsT = V^T block, rhs = Wp^T
                    tensor.wait_ge(sem_dw, 16)
                    for b in range(NKB):
                        vf = 6 + (128 * b) // 1024
                        sname, thr = fillcopy_wait(plan, vf)
                        tensor.wait_ge(sems[sname], thr)
                        if b >= 2:
                            tensor.wait_ge(sem_dve, plan["dve_after_vcopy"][b - 2])
                        half = 512 * (b % 2)
                        tensor.matmul(
                            pV[0:128, half : half + 64],
                            s_VT[0:64, 128 * b : 128 * b + 128],
                            s_wp[0:64, 0:64],
                            start=True,
                            stop=True,
                        ).then_inc(sem_pe, 1)

                @blk.scalar
                def _(scalar):
                    for fi, (kind, lo) in enumerate(FILLS):
                        if fi % 2 != 0:
                            continue
                        scalar.wait_ge(sem_pe, plan["pe_after_fill"][fi])
                        scalar.copy(
                            proj_dst[kind][0:64, lo : lo + 1024],
                            pbuf[fi % 2][0:64, 0:1024],
                        ).then_inc(sem_act, 1)

                @blk.vector
                def _(vector):
                    vector.memset(s_wu[:, :], 0.25).then_inc(sem_dve, 1)
                    vhat_ones = s_vhat[:, :].rearrange("p (b c) -> p b c", c=VST)[
                        :, :, 64:65
                    ]
                    vector.memset(vhat_ones, 1.0).then_inc(sem_dve, 1)
                    vector.memset(s_ones1[:, :], 1.0).then_inc(sem_dve, 1)
                    for fi, (kind, lo) in enumerate(FILLS):
                        if fi % 2 != 1:
                            continue
                        vector.wait_ge(sem_pe, plan["pe_after_fill"][fi])
                        vector.tensor_copy(
                            proj_dst[kind][0:64, lo : lo + 1024],
                            pbuf[fi % 2][0:64, 0:1024],
                        ).then_inc(sem_dve, 1)
                    for b in range(NKB):
                        half = 512 * (b % 2)
                        vector.wait_ge(sem_pe, plan["pe_after_vnat"][b])
                        vector.tensor_copy(
                            s_vhat[0:128, VST * b : VST * b + 64],
                            pV[0:128, half : half + 64],
                        ).then_inc(sem_dve, 1)

        # =============== main attention loop ===============
        with nc.psum_tensor("p_acc", [65, QL], FP) as p_acc:
            with nc.psum_tensor("p_strip", [128, 2048], FP) as p_strip:
                with nc.Block() as blk:

                    @blk.tensor
                    def _(tensor):
                        def emit_pv(pv):
                            tensor.wait_ge(sem_dve, plan["dve_after_mask"][pv])
                            for c0, c1 in chunks(0, nq_of(pv), 512):
                                tensor.matmul(
                                    p_acc[0:65, c0:c1],
                                    s_vhat[0:128, VST * pv : VST * pv + 65],
                                    s_P[pv % 2][0:128, c0:c1],
                                    start=False,
                                    stop=(pv == NKB - 1),
                                    skip_group_check=True,
                                ).then_inc(sem_pe, 1)

                        # Q/K projections must be in SBUF
                        for fi in range(6):
                            sname, thr = fillcopy_wait(plan, fi)
                            tensor.wait_ge(sems[sname], thr)

                        for u, (kb, lo, hi) in enumerate(units):
                            if u >= 2:
                                tensor.wait_ge(sem_act, plan["act_after_unit"][u - 2])
                            base = 1024 * (u % 2)
                            for c0, c1 in chunks(lo, hi, 512):
                                tensor.matmul(
                                    p_strip[0:128, base + c0 - lo : base + c1 - lo],
                                    s_KT[0:64, 128 * kb : 128 * kb + 128],
                                    s_QT[0:64, c0:c1],
                                    start=True,
                                    stop=True,
                                ).then_inc(sem_pe, 1)
                            if (u + 1 == len(units)) or units[u + 1][0] != kb:
                                if kb >= 1:
                                    emit_pv(kb - 1)
                        emit_pv(NKB - 1)

                    @blk.scalar
                    def _(scalar):
                        for u, (kb, lo, hi) in enumerate(units):
                            scalar.wait_ge(sem_pe, plan["pe_after_sunit"][u])
                            base = 1024 * (u % 2)
                            scalar.activation(
                                s_P[kb % 2][0:128, lo:hi],
                                p_strip[0:128, base : base + hi - lo],
                                mybir.ActivationFunctionType.Exp,
                                scale=0.125,
                            ).then_inc(sem_act, 1)
                        # l row (ones-column accumulation) to SBUF
                        scalar.wait_ge(sem_pe, plan["pe_after_pv"][NKB - 1])
                        scalar.copy(s_lrow[0:1, 0:QL], p_acc[64:65, 0:QL]).then_inc(
                            sem_act, 1
                        )


                    @blk.vector
                    def _(vector):
                        vector.memset(p_acc[0:65, 0:QL], 0).then_inc(sem_dve, 1)
                        vector.wait_ge(sem_dm, 48)
                        for kb in range(NKB):
                            u_last = max(
                                u for u, (k2, _, _) in enumerate(units) if k2 == kb
                            )
                            vector.wait_ge(sem_act, plan["act_after_unit"][u_last])
                            s_last = min(kb // 2, NSLOT - 1)
                            c0 = 128 * s_last
                            m = s_me if kb % 2 == 0 else s_mo
                            vector.tensor_mul(
                                s_P[kb % 2][0:128, c0 : c0 + 128],
                                s_P[kb % 2][0:128, c0 : c0 + 128],
                                m[0:128, 0:128],
                            ).then_inc(sem_dve, 1)

            # ========= finale: broadcast l, 1/x on 64 lanes, normalize =========
            with nc.psum_tensor("p_bc", [64, QL], FP) as p_bc:
                with nc.Block() as blk:

                    @blk.sync
                    def _(sync):
                        sync.wait_ge(sem_dve, plan["dve_after_y"])
                        sync.dma_start(d_o[:, :], s_Y[0:64, :]).then_inc(sem_out, 16)
                        sync.wait_ge(sem_out, 16)

                    @blk.vector
                    def _(vector):
                        vector.wait_ge(sem_act, plan["act_after_lrow"])
                        vector.tensor_copy(s_lrowb[0:1, :], s_lrow[0:1, :]).then_inc(
                            sem_dve, 1
                        )
                        vector.wait_ge(sem_act, plan["act_after_recip"])
                        vector.scalar_tensor_tensor(
                            s_Y[0:64, :],
                            p_acc[0:64, 0:QL],
                            0.0,
                            s_rb[0:64, :],
                            op0=mybir.AluOpType.add,
                            op1=mybir.AluOpType.mult,
                        ).then_inc(sem_dve, 1)
                        vector.drain()
                        vector.tensor_scalar_add(
                            s_Y[0:64, :], s_Y[0:64, :], s_bpc[0:64, 0:1]
                        ).then_inc(sem_dve, 1)

                    @blk.tensor
                    def _(tensor):
                        tensor.wait_ge(sem_dve, plan["dve_after_lcast"])
                        for c0, c1 in chunks(0, QL, 512):
                            tensor.matmul(
                                p_bc[0:64, c0:c1],
                                s_ones1[0:1, 0:64],
                                s_lrowb[0:1, c0:c1],
                                start=True,
                                stop=True,
                            ).then_inc(sem_pe, 1)

                    @blk.scalar
                    def _(scalar):
                        # 1/l on the broadcast block (ACT Reciprocal; its known
                        # inaccuracy is far below our tolerance)
                        scalar.wait_ge(sem_pe, plan["pe_after_pv"][NKB - 1] + 4)
                        ins_ = [
                            scalar.lower_ap(p_bc[0:64, 0:QL]),
                            mybir.ImmediateValue(dtype=mybir.dt.float32, value=0.0),
                            mybir.ImmediateValue(dtype=mybir.dt.float32, value=1.0),
                            mybir.ImmediateValue(dtype=mybir.dt.float32, value=0.0),
                        ]
                        scalar.add_instruction(
                            mybir.InstActivation(
                                name=scalar.bass.get_next_instruction_name(),
                                func=mybir.ActivationFunctionType.Reciprocal,
                                ins=ins_,
                                outs=[scalar.lower_ap(s_rb[0:64, 0:QL])],
                            )
                        ).then_inc(sem_act, 1)

    return nc


_NC_CACHE = []


def _get_nc():
    if not _NC_CACHE:
        _NC_CACHE.append(build())
    return _NC_CACHE[0]


def _make_in_maps(q, k, v, Wq, bq, Wk, bk, Wv, bv, Wp, bp):
    f32 = np.float32
    bf16 = mybir.dt.np(BF)

    def prep(x):
        return np.ascontiguousarray(np.asarray(x, dtype=f32))

    q, k, v = prep(q), prep(k), prep(v)
    wq_aug = np.concatenate([prep(Wq).T, prep(bq)[None, :]], axis=0).astype(bf16)
    wk_aug = np.concatenate([prep(Wk).T, prep(bk)[None, :]], axis=0).astype(bf16)
    wv_aug = np.concatenate([prep(Wv).T, prep(bv)[None, :]], axis=0).astype(bf16)
    wp_t = np.ascontiguousarray(prep(Wp).T).astype(bf16)
    bpc = np.ascontiguousarray(prep(bp)[:, None])
    ones1 = np.ones((1,), f32)

    tril = np.tril(np.ones((128, 128), f32)).astype(bf16)  # [c, r] = 1 iff r <= c
    zeros = np.zeros((128, 128), bf16)
    ones_m = np.ones((128, 128), bf16)

    in_maps = []
    for c in range(8):
        b, h = c // 2, c % 2
        qsel = q[b].reshape(32, 128, D)[h::2].reshape(QL, D)
        qta = np.concatenate([qsel.T, np.broadcast_to(ones1, (1, QL))], axis=0)
        kta = np.concatenate([k[b].T, np.broadcast_to(ones1, (1, S))], axis=0)
        vta = np.concatenate([v[b].T, np.broadcast_to(ones1, (1, S))], axis=0)
        in_maps.append(
            {
                "qta": np.ascontiguousarray(qta.astype(bf16)),
                "kta": np.ascontiguousarray(kta.astype(bf16)),
                "vta": np.ascontiguousarray(vta.astype(bf16)),
                "wq": wq_aug,
                "wk": wk_aug,
                "wv": wv_aug,
                "wp": wp_t,
                "bpc": bpc,
                "me": tril if h == 0 else zeros,
                "mo": ones_m if h == 0 else tril,
            }
        )
    return in_maps


def run(inputs, trace=False, **kw):
    nc = _get_nc()
    in_maps = _make_in_maps(**inputs)
    res = run_bass_kernel_spmd(nc, in_maps, core_ids=list(range(8)), trace=trace, **kw)
    out = np.empty((B, S, D), np.float32)
    for c in range(8):
        b, h = c // 2, c % 2
        o = np.asarray(res.results[c]["o"], dtype=np.float32)  # [64, 2048]
        out[b].reshape(32, 128, D)[h::2] = o.T.reshape(NSLOT, 128, D)
    return out, res


def kernel(**inputs):
    out, _ = run(inputs, trace=False)
    return out


# revision 37
# speedup vs baseline: 1.0747x; 1.0747x over previous
"""Trainium2 Bass kernel: single-head attention with QKV+output projections.

Reference math (B=4, S=4096, D=64):
    Q = q@Wq.T+bq; K = k@Wk.T+bk; V = v@Wv.T+bv
    s = (Q @ K.T) / 8, masked -inf where i > j  (query i attends keys j >= i)
    out = softmax(s) @ V @ Wp.T + bp

Sharding (8 cores): core c -> batch b = c//2, parity h = c%2.
Each core handles 16 query tiles of 128 rows: global tiles t = 2s+h
(s = local slot 0..15). For key-block kb (128 keys), the valid local
query prefix is Nq(kb) = 128*min(kb//2+1, 16) columns (slot s valid iff
t <= kb). Scores are computed transposed (S^T: keys on partitions,
queries on free dim) so the PV matmul needs no transposes; the softmax
denominator l rides along as a ones-column appended to V''=V@Wp.T.
Masking of the one partially-valid/invalid last slot per key-block is
data-driven (per-core {0,1} bf16 mask tensors), so all 8 cores share a
single SPMD graph. No max-subtraction in softmax: |s/8| < ~3, exp is
safe in fp32.

All TensorEngine inputs are bf16 (fp32/f32r matmuls and PE transposes
interact badly on HW); PSUM accumulation stays fp32. V-hat natural
layout comes from DMA-transposes (XBAR), not PE transposes. The final
normalize runs in the O''^T layout: 1/l is computed compactly and
row-broadcast via a rank-1 bf16 matmul; the host transposes the
[64, 2048] result back (pure data movement).
"""

import numpy as np

import concourse.bass as bass
import concourse.mybir as mybir
from concourse.bass_utils import run_bass_kernel_spmd

B, S, D = 4, 4096, 64
NSLOT = 16  # query tiles (slots) per core
NKB = 32  # key blocks of 128
QL = NSLOT * 128  # local queries per core = 2048
VST = 80  # vhat block stride (65 used; multiple of 16 for XBAR dma-transpose)
NWARM = 24  # dummy warm-up matmuls to unthrottle the PE HAM clock gate

FP = mybir.dt.float32
BF = mybir.dt.bfloat16

# prep fills: [64,1024] PSUM projection fills; copies alternate ACT/DVE
FILLS = (
    [("q", 1024 * f) for f in range(2)]
    + [("k", 1024 * f) for f in range(4)]
    + [("v", 1024 * f) for f in range(4)]
)


def nq_of(kb: int) -> int:
    """Valid local-query prefix width (cols of S^T) at key-block kb."""
    if kb < 0:
        return 0
    return 128 * min(kb // 2 + 1, NSLOT)


def chunks(lo: int, hi: int, step: int):
    out = []
    c = lo
    while c < hi:
        out.append((c, min(c + step, hi)))
        c = out[-1][1]
    return out


def make_units():
    units = []
    for kb in range(NKB):
        w = nq_of(kb)
        if w <= 1024:
            units.append((kb, 0, w))
        else:
            units.append((kb, 0, 1024))
            units.append((kb, 1024, w))
    return units


def make_plan():
    """Precompute per-engine cumulative instruction counts at key events."""
    units = make_units()
    p = {
        "units": units,
        "pe_after_fill": {},
        "pe_after_sunit": {},
        "pe_after_pv": {},
        "act_after_fillcopy": {},
        "dve_after_fillcopy": {},
        "act_after_unit": {},
        "dve_after_mask": {},
    }

    # --- PE --- HAM warm-up burst, fills, then S/PV loop, bcast matmuls
    pe = NWARM
    for fi in range(len(FILLS)):
        pe += 2
        p["pe_after_fill"][fi] = pe
    for b in range(NKB):
        pe += 1
        p.setdefault("pe_after_vnat", {})[b] = pe
    for u, (kb, lo, hi) in enumerate(units):
        pe += len(chunks(lo, hi, 512))
        p["pe_after_sunit"][u] = pe
        if (u + 1 == len(units)) or units[u + 1][0] != kb:
            pv = kb - 1
            if pv >= 0:
                pe += len(chunks(0, nq_of(pv), 512))
                p["pe_after_pv"][pv] = pe
    pe += len(chunks(0, nq_of(NKB - 1), 512))
    p["pe_after_pv"][NKB - 1] = pe

    # --- ACT --- even fill copies, exps, l-row copy
    act = 0
    for fi in range(0, len(FILLS), 2):
        act += 1
        p["act_after_fillcopy"][fi] = act
    for u in range(len(units)):
        act += 1
        p["act_after_unit"][u] = act
    act += 1
    p["act_after_lrow"] = act
    act += 1  # ACT reciprocal of the broadcast l
    p["act_after_recip"] = act

    # --- DVE --- memsets, odd fill copies, vhat copies, p_acc memset, masks
    dve = 3  # warmup-scratch memset + vhat-ones memset + ones164 memset
    for fi in range(1, len(FILLS), 2):
        dve += 1
        p["dve_after_fillcopy"][fi] = dve
    for b in range(NKB):
        dve += 1
        p.setdefault("dve_after_vcopy", {})[b] = dve
    dve += 1  # p_acc zero-init
    p["dve_acc_memset"] = dve
    for kb in range(NKB):
        dve += 1
        p["dve_after_mask"][kb] = dve
    dve += 1  # l row f32 -> bf16 cast
    p["dve_after_lcast"] = dve
    dve += 2  # stt, bp add
    p["dve_after_y"] = dve

    return p


def fillcopy_wait(plan, fi):
    """(sem_name, threshold) completing the copy-out of fill fi."""
    if fi % 2 == 0:
        return ("act", plan["act_after_fillcopy"][fi])
    return ("dve", plan["dve_after_fillcopy"][fi])


def build():
    plan = make_plan()
    units = plan["units"]

    nc = bass.Bass()

    # ---- DRAM parameters (per-core shards, host-prepared, bf16) ----
    d_qta = nc.declare_dram_parameter("qta", [65, QL], BF, isOutput=False)
    d_kta = nc.declare_dram_parameter("kta", [65, S], BF, isOutput=False)
    d_vta = nc.declare_dram_parameter("vta", [65, S], BF, isOutput=False)
    d_wq = nc.declare_dram_parameter("wq", [65, 64], BF, isOutput=False)
    d_wk = nc.declare_dram_parameter("wk", [65, 64], BF, isOutput=False)
    d_wv = nc.declare_dram_parameter("wv", [65, 64], BF, isOutput=False)
    d_wp = nc.declare_dram_parameter("wp", [64, 64], BF, isOutput=False)
    d_bpc = nc.declare_dram_parameter("bpc", [64, 1], FP, isOutput=False)
    d_me = nc.declare_dram_parameter("me", [128, 128], BF, isOutput=False)
    d_mo = nc.declare_dram_parameter("mo", [128, 128], BF, isOutput=False)
    d_o = nc.declare_dram_parameter("o", [64, QL], FP, isOutput=True)


    # ---- persistent SBUF ----
    s_qta = nc.alloc_sbuf_tensor("s_qta", [65, QL], BF)
    s_kta = nc.alloc_sbuf_tensor("s_kta", [65, S], BF)
    s_vta = nc.alloc_sbuf_tensor("s_vta", [65, S], BF)
    s_wq = nc.alloc_sbuf_tensor("s_wq", [65, 64], BF)
    s_wk = nc.alloc_sbuf_tensor("s_wk", [65, 64], BF)
    s_wv = nc.alloc_sbuf_tensor("s_wv", [65, 64], BF)
    s_wp = nc.alloc_sbuf_tensor("s_wp", [64, 64], BF)
    s_bpc = nc.alloc_sbuf_tensor("s_bpc", [64, 1], FP)
    s_me = nc.alloc_sbuf_tensor("s_me", [128, 128], BF)
    s_mo = nc.alloc_sbuf_tensor("s_mo", [128, 128], BF)
    s_QT = nc.alloc_sbuf_tensor("s_QT", [64, QL], BF)
    s_KT = nc.alloc_sbuf_tensor("s_KT", [64, S], BF)
    s_VT = nc.alloc_sbuf_tensor("s_VT", [64, S], BF)
    s_vhat = nc.alloc_sbuf_tensor("s_vhat", [128, NKB * VST], BF)
    s_P = [
        nc.alloc_sbuf_tensor("s_P0", [128, QL], BF),
        nc.alloc_sbuf_tensor("s_P1", [128, QL], BF),
    ]
    s_lrow = nc.alloc_sbuf_tensor("s_lrow", [1, QL], FP)
    s_lrowb = nc.alloc_sbuf_tensor("s_lrowb", [1, QL], BF)
    s_scr = nc.alloc_sbuf_tensor("s_scr", [64, QL], FP)
    s_ones1 = nc.alloc_sbuf_tensor("s_ones1", [1, 64], BF)
    s_rb = nc.alloc_sbuf_tensor("s_rb", [64, QL], FP)
    s_Y = nc.alloc_sbuf_tensor("s_Y", [64, QL], FP)
    s_wu = nc.alloc_sbuf_tensor("s_wu", [64, 640], BF)

    proj_dst = {"q": s_QT, "k": s_KT, "v": s_VT}

    with (
        nc.semaphore("sem_dq") as sem_dq,
        nc.semaphore("sem_dk") as sem_dk,
        nc.semaphore("sem_dv") as sem_dv,
        nc.semaphore("sem_dw") as sem_dw,
        nc.semaphore("sem_dm") as sem_dm,
        nc.semaphore("sem_pe") as sem_pe,
        nc.semaphore("sem_act") as sem_act,
        nc.semaphore("sem_dve") as sem_dve,
        nc.semaphore("sem_out") as sem_out,
    ):
        sems = {"act": sem_act, "dve": sem_dve}

        # =============== input DMAs ===============
        with nc.Block() as blk:

            @blk.sync
            def _(sync):
                groups = [
                    (sem_dq, [(d_wq, s_wq), (d_qta, s_qta)]),
                    (sem_dk, [(d_wk, s_wk), (d_kta, s_kta)]),
                    (sem_dv, [(d_wv, s_wv), (d_vta, s_vta)]),
                    (sem_dw, [(d_wp, s_wp)]),
                    (sem_dm, [(d_bpc, s_bpc), (d_me, s_me), (d_mo, s_mo)]),
                ]
                for sem, pairs in groups:
                    for dsrc, sdst in pairs:
                        sync.dma_start(sdst[:, :], dsrc[:, :]).then_inc(sem, 16)

        # =============== prep stage ===============
        with (
            nc.psum_tensor("pA", [64, 1024], FP) as pA,
            nc.psum_tensor("pB", [64, 1024], FP) as pB,
            nc.psum_tensor("pV", [128, 1024], FP) as pV,
        ):
            pbuf = [pA, pB]

            with nc.Block() as blk:

                @blk.tensor
                def _(tensor):
                    # HAM warm-up: ~10us of back-to-back matmuls on scratch
                    # data while the input DMAs are in flight. Keeps the PE
                    # activity window busy so the clock gate opens (1.2 ->
                    # 2.4 GHz) before the real work arrives.
                    tensor.wait_ge(sem_dve, 1)
                    for _ in range(NWARM):
                        tensor.matmul(
                            pA[0:64, 0:512],
                            s_wu[0:64, 0:64],
                            s_wu[0:64, 128:640],
                            start=True,
                            stop=True,
                        ).then_inc(sem_pe, 1)
                    for fi, (kind, lo) in enumerate(FILLS):
                        buf = pbuf[fi % 2]
                        if fi >= 2:
                            sname, thr = fillcopy_wait(plan, fi - 2)
                            tensor.wait_ge(sems[sname], thr)
                        if kind == "q":
                            tensor.wait_ge(sem_dq, 32)
                            w_ap, src, cdim = s_wq, s_qta, 65
                        elif kind == "k":
                            tensor.wait_ge(sem_dk, 32)
                            w_ap, src, cdim = s_wk, s_kta, 65
                        else:
                            tensor.wait_ge(sem_dv, 32)
                            w_ap, src, cdim = s_wv, s_vta, 65
                        for c0, c1 in chunks(lo, lo + 1024, 512):
                            tensor.matmul(
                                buf[0:64, c0 - lo : c1 - lo],
                                w_ap[0:cdim, 0:64],
                                src[0:cdim, c0:c1],
                                start=True,
                                stop=True,
                            ).then_inc(sem_pe, 1)

                    # V''-natural tiles: lhsT = V^T block, rhs = Wp^T
                    tensor.wait_ge(sem_dw, 16)
                    for b in range(NKB):
                        vf = 6 + (128 * b) // 1024
                        sname, thr = fillcopy_wait(plan, vf)
                        tensor.wait_ge(sems[sname], thr)
                        if b >= 2:
                            tensor.wait_ge(sem_dve, plan["dve_after_vcopy"][b - 2])
                        half = 512 * (b % 2)
                        tensor.matmul(
                            pV[0:128, half : half + 64],
                            s_VT[0:64, 128 * b : 128 * b + 128],
                            s_wp[0:64, 0:64],
                            start=True,
                            stop=True,
                        ).then_inc(sem_pe, 1)

                @blk.scalar
                def _(scalar):
                    for fi, (kind, lo) in enumerate(FILLS):
                        if fi % 2 != 0:
                            continue
                        scalar.wait_ge(sem_pe, plan["pe_after_fill"][fi])
                        scalar.copy(
                            proj_dst[kind][0:64, lo : lo + 1024],
                            pbuf[fi % 2][0:64, 0:1024],
                        ).then_inc(sem_act, 1)

                @blk.vector
                def _(vector):
                    vector.memset(s_wu[:, :], 0.25).then_inc(sem_dve, 1)
                    vhat_ones = s_vhat[:, :].rearrange("p (b c) -> p b c", c=VST)[
                        :, :, 64:65
                    ]
                    vector.memset(vhat_ones, 1.0).then_inc(sem_dve, 1)
                    vector.memset(s_ones1[:, :], 1.0).then_inc(sem_dve, 1)
                    for fi, (kind, lo) in enumerate(FILLS):
                        if fi % 2 != 1:
                            continue
                        vector.wait_ge(sem_pe, plan["pe_after_fill"][fi])
                        vector.tensor_copy(
                            proj_dst[kind][0:64, lo : lo + 1024],
                            pbuf[fi % 2][0:64, 0:1024],
                        ).then_inc(sem_dve, 1)
                    for b in range(NKB):
                        half = 512 * (b % 2)
                        vector.wait_ge(sem_pe, plan["pe_after_vnat"][b])
                        vector.tensor_copy(
                            s_vhat[0:128, VST * b : VST * b + 64],
                            pV[0:128, half : half + 64],
                        ).then_inc(sem_dve, 1)

        # =============== main attention loop ===============
        with nc.psum_tensor("p_acc", [65, QL], FP) as p_acc:
            with nc.psum_tensor("p_strip", [128, 2048], FP) as p_strip:
                with nc.Block() as blk:

                    @blk.tensor
                    def _(tensor):
                        def emit_pv(pv):
                            tensor.wait_ge(sem_dve, plan["dve_after_mask"][pv])
                            for c0, c1 in chunks(0, nq_of(pv), 512):
                                tensor.matmul(
                                    p_acc[0:65, c0:c1],
                                    s_vhat[0:128, VST * pv : VST * pv + 65],
                                    s_P[pv % 2][0:128, c0:c1],
                                    start=False,
                                    stop=(pv == NKB - 1),
                                    skip_group_check=True,
                                ).then_inc(sem_pe, 1)

                        # Q/K projections must be in SBUF
                        for fi in range(6):
                            sname, thr = fillcopy_wait(plan, fi)
                            tensor.wait_ge(sems[sname], thr)

                        for u, (kb, lo, hi) in enumerate(units):
                            if u >= 2:
                                tensor.wait_ge(sem_act, plan["act_after_unit"][u - 2])
                            base = 1024 * (u % 2)
                            for c0, c1 in chunks(lo, hi, 512):
                                tensor.matmul(
                                    p_strip[0:128, base + c0 - lo : base + c1 - lo],
                                    s_KT[0:64, 128 * kb : 128 * kb + 128],
                                    s_QT[0:64, c0:c1],
                                    start=True,
                                    stop=True,
                                ).then_inc(sem_pe, 1)
                            if (u + 1 == len(units)) or units[u + 1][0] != kb:
                                if kb >= 1:
                                    emit_pv(kb - 1)
                        emit_pv(NKB - 1)

                    @blk.scalar
                    def _(scalar):
                        for u, (kb, lo, hi) in enumerate(units):
                            scalar.wait_ge(sem_pe, plan["pe_after_sunit"][u])
                            base = 1024 * (u % 2)
                            scalar.activation(
                                s_P[kb % 2][0:128, lo:hi],
                                p_strip[0:128, base : base + hi - lo],
                                mybir.ActivationFunctionType.Exp,
                                scale=0.125,
                            ).then_inc(sem_act, 1)
                        # l row (ones-column accumulation) to SBUF
                        scalar.wait_ge(sem_pe, plan["pe_after_pv"][NKB - 1])
                        scalar.copy(s_lrow[0:1, 0:QL], p_acc[64:65, 0:QL]).then_inc(
                            sem_act, 1
                        )


                    @blk.vector
                    def _(vector):
                        vector.memset(p_acc[0:65, 0:QL], 0).then_inc(sem_dve, 1)
                        vector.wait_ge(sem_dm, 48)
                        for kb in range(NKB):
                            u_last = max(
                                u for u, (k2, _, _) in enumerate(units) if k2 == kb
                            )
                            vector.wait_ge(sem_act, plan["act_after_unit"][u_last])
                            s_last = min(kb // 2, NSLOT - 1)
                            c0 = 128 * s_last
                            m = s_me if kb % 2 == 0 else s_mo
                            vector.tensor_mul(
                                s_P[kb % 2][0:128, c0 : c0 + 128],
                                s_P[kb % 2][0:128, c0 : c0 + 128],
                                m[0:128, 0:128],
                            ).then_inc(sem_dve, 1)

            # ========= finale: broadcast l, 1/x on 64 lanes, normalize =========
            with nc.psum_tensor("p_bc", [64, QL], FP) as p_bc:
                with nc.Block() as blk:

                    @blk.sync
                    def _(sync):
                        sync.wait_ge(sem_dve, plan["dve_after_y"])
                        sync.dma_start(d_o[:, :], s_Y[0:64, :]).then_inc(sem_out, 16)
                        sync.wait_ge(sem_out, 16)

                    @blk.vector
                    def _(vector):
                        vector.wait_ge(sem_act, plan["act_after_lrow"])
                        vector.tensor_copy(s_lrowb[0:1, :], s_lrow[0:1, :]).then_inc(
                            sem_dve, 1
                        )
                        vector.wait_ge(sem_act, plan["act_after_recip"])
                        vector.scalar_tensor_tensor(
                            s_Y[0:64, :],
                            p_acc[0:64, 0:QL],
                            0.0,
                            s_rb[0:64, :],
                            op0=mybir.AluOpType.add,
                            op1=mybir.AluOpType.mult,
                        ).then_inc(sem_dve, 1)
                        vector.drain()
                        vector.tensor_scalar_add(
                            s_Y[0:64, :], s_Y[0:64, :], s_bpc[0:64, 0:1]
                        ).then_inc(sem_dve, 1)

                    @blk.tensor
                    def _(tensor):
                        tensor.wait_ge(sem_dve, plan["dve_after_lcast"])
                        for c0, c1 in chunks(0, QL, 512):
                            tensor.matmul(
                                p_bc[0:64, c0:c1],
                                s_ones1[0:1, 0:64],
                                s_lrowb[0:1, c0:c1],
                                start=True,
                                stop=True,
                            ).then_inc(sem_pe, 1)

                    @blk.scalar
                    def _(scalar):
                        # 1/l on the broadcast block (ACT Reciprocal; its known
                        # inaccuracy is far below our tolerance)
                        scalar.wait_ge(sem_pe, plan["pe_after_pv"][NKB - 1] + 4)
                        ins_ = [
                            scalar.lower_ap(p_bc[0:64, 0:QL]),
                            mybir.ImmediateValue(dtype=mybir.dt.float32, value=0.0),
                            mybir.ImmediateValue(dtype=mybir.dt.float32, value=1.0),
                            mybir.ImmediateValue(dtype=mybir.dt.float32, value=0.0),
                        ]
                        scalar.add_instruction(
                            mybir.InstActivation(
                                name=scalar.bass.get_next_instruction_name(),
                                func=mybir.ActivationFunctionType.Reciprocal,
                                ins=ins_,
                                outs=[scalar.lower_ap(s_rb[0:64, 0:QL])],
                            )
                        ).then_inc(sem_act, 1)

    return nc


_NC_CACHE = []


def _get_nc():
    if not _NC_CACHE:
        _NC_CACHE.append(build())
    return _NC_CACHE[0]


def _make_in_maps(q, k, v, Wq, bq, Wk, bk, Wv, bv, Wp, bp):
    f32 = np.float32
    bf16 = mybir.dt.np(BF)

    def prep(x):
        return np.ascontiguousarray(np.asarray(x, dtype=f32))

    q, k, v = prep(q), prep(k), prep(v)
    wq_aug = np.concatenate([prep(Wq).T, prep(bq)[None, :]], axis=0).astype(bf16)
    wk_aug = np.concatenate([prep(Wk).T, prep(bk)[None, :]], axis=0).astype(bf16)
    wv_aug = np.concatenate([prep(Wv).T, prep(bv)[None, :]], axis=0).astype(bf16)
    wp_t = np.ascontiguousarray(prep(Wp).T).astype(bf16)
    bpc = np.ascontiguousarray(prep(bp)[:, None])
    ones1 = np.ones((1,), f32)

    tril = np.tril(np.ones((128, 128), f32)).astype(bf16)  # [c, r] = 1 iff r <= c
    zeros = np.zeros((128, 128), bf16)
    ones_m = np.ones((128, 128), bf16)

    in_maps = []
    for c in range(8):
        b, h = c // 2, c % 2
        qsel = q[b].reshape(32, 128, D)[h::2].reshape(QL, D)
        qta = np.concatenate([qsel.T, np.broadcast_to(ones1, (1, QL))], axis=0)
        kta = np.concatenate([k[b].T, np.broadcast_to(ones1, (1, S))], axis=0)
        vta = np.concatenate([v[b].T, np.broadcast_to(ones1, (1, S))], axis=0)
        in_maps.append(
            {
                "qta": np.ascontiguousarray(qta.astype(bf16)),
                "kta": np.ascontiguousarray(kta.astype(bf16)),
                "vta": np.ascontiguousarray(vta.astype(bf16)),
                "wq": wq_aug,
                "wk": wk_aug,
                "wv": wv_aug,
                "wp": wp_t,
                "bpc": bpc,
                "me": tril if h == 0 else zeros,
                "mo": ones_m if h == 0 else tril,
            }
        )
    return in_maps


def run(inputs, trace=False, **kw):
    nc = _get_nc()
    in_maps = _make_in_maps(**inputs)
    res = run_bass_kernel_spmd(nc, in_maps, core_ids=list(range(8)), trace=trace, **kw)
    out = np.empty((B, S, D), np.float32)
    for c in range(8):
        b, h = c // 2, c % 2
        o = np.asarray(res.results[c]["o"], dtype=np.float32)  # [64, 2048]
        out[b].reshape(32, 128, D)[h::2] = o.T.reshape(NSLOT, 128, D)
    return out, res


def kernel(**inputs):
    out, _ = run(inputs, trace=False)
    return out
